# revision 7
# baseline (speedup 1.0000x reference)
"""Trainium2 Bass kernel for DigitConvolutionalModel.

Math: the 3x3 valid conv on the 28x28 image is a linear map, so it folds into
the first Linear layer:
    out = relu(x @ W_eff + b1) @ w2.T + b2
where W_eff[784, 128] = C @ w1.T and C[784, 676] is the conv-as-matrix built
from conv_w.  W_eff is built on the host (O(1) w.r.t. batch); the device does
the two batch matmuls.

Distribution: pure data parallel — batch dim of x sharded across 8 NeuronCores,
weights replicated.  Each core computes out.T [10, 8192]; the host reassembles
[65536, 10].

dtypes: x ships as float8e3 (e3m4: 4 mantissa bits), scaled by 2 on the host so
N(0,1) data sits in the normal range (max |2x| ~ 11 < 15.5); the 1/2 is folded
into the fp16 weights.  Measured end-to-end rel-max error 1.3e-2 vs the 2e-2
gate.  fp8 halves HBM traffic (~6.4 MB/core), putting the DMA stream (~18us)
safely under the PE's ~28us of matmul work, so the tensor engine never starves
and holds its warm 2.4 GHz clock.

Schedule: k-outer across 2 passes of 8 batch tiles.  All 8 PSUM banks act as
accumulators for one pass (bank t <- batch tile t); for each of the 6 main
k-tiles the same stationary weight serves 8 consecutive matmuls.  The 16
remainder features (784 = 6*128 + 16) are one K=16 accumulation matmul per
tile.  Epilogue per tile: relu(+b1) on the ACT engine (fp16 out), second-layer
matmul [10,512] into the just-freed PSUM bank, +b2 on the DVE into a per-pass
output strip stored with a single DMA.
"""

import numpy as np
import ml_dtypes

import concourse.bass as bass  # noqa: F401  (bass registers mybir lowerings)
import concourse.mybir as mybir
import concourse.tile as tile
from concourse import bacc
from concourse.bass_utils import run_bass_kernel_spmd

N_CORES = 8
B = 65536
B_SH = B // N_CORES  # 8192 rows per core
D = 784              # 28*28 input features
DM = 768             # features in the main 128-partition stream
DR = D - DM          # 16 remainder features
H = 128              # hidden
OUT = 10
KT = 128             # contraction tile = full partition dim
NK = DM // KT        # 6 main K-tiles
NB = 512             # batch columns per tile (= one fp32 PSUM bank)
NP = 2               # passes
TPP = 8              # batch tiles per pass (= PSUM banks)
NWARM = 8            # PE clock warm-up matmuls

_CACHE = {}


def _build_nc():
    f32 = mybir.dt.float32
    f16 = mybir.dt.float16
    f8 = mybir.dt.float8e3
    nc = bacc.Bacc("TRN2", target_bir_lowering=False, debug=False,
                   num_devices=N_CORES)
    # main x, partition-major: [p, pass, k, t, c]; feature f = k*128 + p,
    # batch b = pass*4096 + t*512 + c.  Per (pass,k) DMA: 4 KB/partition runs.
    xk = nc.dram_tensor("xk", [KT, NP, NK, TPP, NB], f8,
                        kind="ExternalInput").ap()
    # remainder features 768..784: [p, batch]
    xrem = nc.dram_tensor("xrem", [DR, B_SH], f8, kind="ExternalInput").ap()
    # weights pre-arranged host-side: wk[p, k, m] = W_eff[k*128+p, m] / 2
    wk = nc.dram_tensor("wk", [KT, NK, H], f16, kind="ExternalInput").ap()
    wr = nc.dram_tensor("wr", [DR, H], f16, kind="ExternalInput").ap()
    w2t = nc.dram_tensor("w2t", [H, OUT], f16, kind="ExternalInput").ap()
    b1c = nc.dram_tensor("b1c", [H, 1], f32, kind="ExternalInput").ap()
    b2c = nc.dram_tensor("b2c", [OUT, 1], f32, kind="ExternalInput").ap()
    out = nc.dram_tensor("out", [OUT, B_SH], f32, kind="ExternalOutput").ap()

    with tile.TileContext(nc) as tc:
        with (
            tc.tile_pool(name="wpool", bufs=1) as wpool,
            tc.tile_pool(name="xpool", bufs=NP * NK) as xpool,
            tc.tile_pool(name="hpool", bufs=6) as hpool,
            tc.tile_pool(name="opool", bufs=NP) as opool,
            tc.tile_pool(name="ps", bufs=8, space="PSUM") as pspool,
        ):
            # Params + remainder features ride the scalar ring; the x stream
            # runs on the sync ring so its first trigger issues at t=0.
            wk_sb = wpool.tile([KT, NK, H], f16)
            nc.scalar.dma_start(wk_sb[:], wk[:])
            wr_sb = wpool.tile([DR, H], f16)
            nc.scalar.dma_start(wr_sb[:], wr[:])
            w2_sb = wpool.tile([H, OUT], f16)
            nc.scalar.dma_start(w2_sb[:], w2t[:])
            b1_sb = wpool.tile([H, 1], f32)
            nc.scalar.dma_start(b1_sb[:], b1c[:])
            b2_sb = wpool.tile([OUT, 1], f32)
            nc.scalar.dma_start(b2_sb[:], b2c[:])
            xr_sb = wpool.tile([DR, B_SH], f8)
            nc.scalar.dma_start(xr_sb[:], xrem[:])

            # All 12 x-block DMAs up front on the sync ring, in consumption
            # order; 12 x 512 KB resident (SBUF has room), so no buffer-reuse
            # stalls and the DMA engines stay ahead of the PE throughout.
            x_sb = []
            for pa in range(NP):
                for k in range(NK):
                    t_ = xpool.tile([KT, TPP * NB], f8)
                    nc.sync.dma_start(
                        t_[:], xk[:, pa, k, :, :].rearrange("p t c -> p (t c)"))
                    x_sb.append(t_)

            # PE clock warm-up: HAM reaches 2.4 GHz after ~3.4us of activity.
            # Warm on the weight tile (lands ~1.5us in) so the dense stream
            # starts as early as possible; results land in bank 0, discarded.
            warm_ps = pspool.tile([H, NB], f32, name="acc")
            for _ in range(NWARM):
                nc.tensor.matmul(warm_ps[:], lhsT=wk_sb[:, 0, :],
                                 rhs=wk_sb[:, 0:4, :].rearrange("p a b -> p (a b)"),
                                 start=True, stop=True)

            def epilogue(pa, t, ps1, o_sb):
                # h = relu(ps + b1), alternating ACT/DVE so the per-tile
                # chain pipelines at ~350ns effective; fp16 out.
                h_sb = hpool.tile([H, NB], f16)
                if t % 2 == 0:
                    nc.scalar.activation(
                        h_sb[:], ps1[:],
                        mybir.ActivationFunctionType.Relu, bias=b1_sb[:])
                else:
                    nc.vector.tensor_scalar(
                        h_sb[:], ps1[:], b1_sb[:], 0.0,
                        mybir.AluOpType.add, mybir.AluOpType.max)
                # out.T[10, 512] = w2 @ h.T into the just-freed bank
                ps2 = pspool.tile([OUT, NB], f32, name="acc")
                nc.tensor.matmul(ps2[:], lhsT=w2_sb[:], rhs=h_sb[:],
                                 start=True, stop=True)
                # +b2 on the opposite engine into the pass output strip
                if t % 2 == 0:
                    nc.vector.tensor_scalar_add(
                        o_sb[:, t * NB:(t + 1) * NB], ps2[:], b2_sb[:])
                else:
                    nc.scalar.activation(
                        o_sb[:, t * NB:(t + 1) * NB], ps2[:],
                        mybir.ActivationFunctionType.Identity, bias=b2_sb[:])

            for pa in range(NP):
                ps_t = [pspool.tile([H, NB], f32, name="acc")
                        for t in range(TPP)]
                # k-outer: one stationary weight feeds 8 consecutive matmuls
                for k in range(NK - 1):
                    xs = x_sb[pa * NK + k]
                    for t in range(TPP):
                        nc.tensor.matmul(
                            ps_t[t][:],
                            lhsT=wk_sb[:, k, :],
                            rhs=xs[:, t * NB:(t + 1) * NB],
                            start=(k == 0),
                            stop=False,
                        )
                # last main k-tile: finish tiles one at a time so epilogues
                # overlap the remaining matmuls instead of queueing at the end
                xs = x_sb[pa * NK + NK - 1]
                for t in range(TPP):
                    nc.tensor.matmul(
                        ps_t[t][:], lhsT=wk_sb[:, NK - 1, :],
                        rhs=xs[:, t * NB:(t + 1) * NB],
                        start=False, stop=False,
                    )
                o_sb = opool.tile([OUT, TPP * NB], f32)
                pending = None
                for t in range(TPP):
                    b0 = (pa * TPP + t) * NB
                    nc.tensor.matmul(
                        ps_t[t][:], lhsT=wr_sb[:],
                        rhs=xr_sb[:, b0:b0 + NB],
                        start=False, stop=True,
                    )
                    # one-tile pipeline offset: tile t's epilogue is emitted
                    # after rem(t+1) so the PE never waits on a fresh relu
                    if pending is not None:
                        epilogue(pa, pending, ps_t[pending], o_sb)
                    pending = t
                epilogue(pa, pending, ps_t[pending], o_sb)
                # one store per pass on the sync ring (idle by then)
                nc.sync.dma_start(
                    out[:, pa * TPP * NB:(pa + 1) * TPP * NB], o_sb[:])

    nc.compile()
    return nc


def _get_nc():
    if "nc" not in _CACHE:
        _CACHE["nc"] = _build_nc()
    return _CACHE["nc"]


def _fold_weights(conv_w: np.ndarray, w1: np.ndarray) -> np.ndarray:
    """W_eff[784, 128]: h_pre = x @ W_eff  ==  conv(x) @ w1.T  (float64 accum)."""
    w1k = w1.reshape(H, 26, 26).transpose(1, 2, 0).astype(np.float64)  # [i,j,k]
    cw = conv_w.astype(np.float64)
    W = np.zeros((28, 28, H), np.float64)
    for di in range(3):
        for dj in range(3):
            W[di:di + 26, dj:dj + 26, :] += cw[di, dj] * w1k
    return W.reshape(D, H).astype(np.float32)


def make_in_maps(x, conv_w, w1, b1, w2, b2):
    f8 = ml_dtypes.float8_e3m4
    x = np.asarray(x, np.float32)
    weff = _fold_weights(np.asarray(conv_w, np.float32),
                         np.asarray(w1, np.float32)) * 0.5  # absorb x*2
    # wk[p, k, m] = weff[k*128+p, m]
    wk = np.ascontiguousarray(
        weff[:DM].reshape(NK, KT, H).transpose(1, 0, 2)).astype(np.float16)
    wr = np.ascontiguousarray(weff[DM:]).astype(np.float16)
    w2t = np.ascontiguousarray(np.asarray(w2, np.float32).T).astype(np.float16)
    b1c = np.ascontiguousarray(np.asarray(b1, np.float32).reshape(H, 1))
    b2c = np.ascontiguousarray(np.asarray(b2, np.float32).reshape(OUT, 1))
    in_maps = []
    for i in range(N_CORES):
        xs = (x[i * B_SH:(i + 1) * B_SH] * 2.0).astype(f8)  # [8192, 784]
        # main: [pass*4096 + t*512 + c, k*128 + p] -> [p, pass, k, t, c]
        xk = np.ascontiguousarray(
            xs[:, :DM].reshape(NP, TPP, NB, NK, KT).transpose(4, 0, 3, 1, 2))
        xrem = np.ascontiguousarray(xs[:, DM:].T)           # [16, 8192]
        in_maps.append({"xk": xk, "xrem": xrem, "wk": wk, "wr": wr,
                        "w2t": w2t, "b1c": b1c, "b2c": b2c})
    return in_maps


def kernel(x, conv_w, w1, b1, w2, b2):
    nc = _get_nc()
    in_maps = make_in_maps(x, conv_w, w1, b1, w2, b2)
    res = run_bass_kernel_spmd(nc, in_maps, list(range(N_CORES)))
    out = np.concatenate([res.results[i]["out"] for i in range(N_CORES)], axis=1)
    return np.ascontiguousarray(out.T)  # [65536, 10] float32


# revision 8
# speedup vs baseline: 1.0247x; 1.0247x over previous
"""Trainium2 Bass kernel for DigitConvolutionalModel.

Math: the 3x3 valid conv on the 28x28 image is a linear map, so it folds into
the first Linear layer:
    out = relu(x @ W_eff + b1) @ w2.T + b2
where W_eff[784, 128] = C @ w1.T and C[784, 676] is the conv-as-matrix built
from conv_w.  W_eff is built on the host (O(1) w.r.t. batch); the device does
the two batch matmuls.

Distribution: pure data parallel — batch dim of x sharded across 8 NeuronCores,
weights replicated.  Each core computes out.T [10, 8192]; the host reassembles
[65536, 10].

dtypes: x ships as float8e3 (e3m4: 4 mantissa bits), scaled by 2 on the host so
N(0,1) data sits in the normal range; the 1/2 is folded into the fp16 weights.
Measured end-to-end rel-max error 1.28e-2 vs the 2e-2 gate.  fp8 halves HBM
traffic (~6.4 MB/core) so the two DMA rings feed the PE with ~2x margin and
the tensor engine never starves (any PE idle gap makes the HAM down-clock the
2.4 GHz PE to half duty for ~7-14us — density is everything).

Schedule: k-outer across 2 passes of 8 batch tiles.  All 8 PSUM banks act as
accumulators for one pass (bank t <- batch tile t); for each of the 6 main
k-tiles one stationary weight serves 8 consecutive matmuls (the redundant
per-matmul LDWEIGHTS are stripped from the compiled IR — the ISA matmul uses
the currently-loaded weights).  The 16 remainder features (784 = 6*128 + 16)
are one K=16 matmul per tile, interleaved with a 2-tile-deep epilogue software
pipeline: relu(+b1) alternating ACT/DVE (fp16 h), second-layer matmul [10,512]
into the just-freed PSUM bank, +b2 on the opposite engine, one output store
per pass.

Rings: x blocks alternate between the sync and scalar HW DGE rings (one ring
sustains only ~280 GB/s; two give ~2x margin over the PE's consumption rate).
Params ride gpsimd's software DGE, which comes up before the HW rings' ~8us
cold start, so the weights beat the first x block.  Warm-up matmuls on a
memset tile (no DMA dependency) keep the PE busy from queue boot until real
data lands, holding the HAM at full clock.
"""

import numpy as np
import ml_dtypes

import concourse.bass as bass  # noqa: F401  (bass registers mybir lowerings)
import concourse.mybir as mybir
import concourse.tile as tile
from concourse import bacc
from concourse.bass_utils import run_bass_kernel_spmd

N_CORES = 8
B = 65536
B_SH = B // N_CORES  # 8192 rows per core
D = 784              # 28*28 input features
DM = 768             # features in the main 128-partition stream
DR = D - DM          # 16 remainder features
H = 128              # hidden
OUT = 10
KT = 128             # contraction tile = full partition dim
NK = DM // KT        # 6 main K-tiles
NB = 512             # batch columns per tile (= one fp32 PSUM bank)
NP = 2               # passes
TPP = 8              # batch tiles per pass (= PSUM banks)
NWARM = 10           # PE clock warm-up matmuls

_CACHE = {}


def _strip_redundant_ldweights(nc):
    """Drop back-to-back InstLdweights with identical operands.

    Legalization emits one LDWEIGHTS per matmul even when consecutive matmuls
    share the stationary operand; the ISA matmul uses the currently-loaded
    weights, so repeats are pure overhead (~50-100ns each on the PE queue).
    Keep any carrying a semaphore wait or with dependents.
    """
    import re
    dep_names = set()
    for f in nc.m.functions:
        for b in f.blocks:
            for i in b.instructions:
                dep_names.update(i.sync_dependency_names())
                dep_names.update(i.nosync_dependency_names())
    n_drop = 0
    for f in nc.m.functions:
        for b in f.blocks:
            insts = list(b.instructions)
            keep = []
            last_sig = None
            for i in insts:
                if type(i).__name__ == 'InstLdweights':
                    c = i.concise if isinstance(i.concise, str) else i.concise()
                    m = re.search(r'in=\[.*?\] tile_size=\S+ tile_position=\S+', c)
                    sig = m.group(0) if m else None
                    if (sig is not None and sig == last_sig
                            and 'wait:' not in c and i.name not in dep_names):
                        n_drop += 1
                        continue
                    last_sig = sig
                keep.append(i)
            if len(keep) != len(insts):
                b.instructions = keep
    return n_drop


def _build_nc():
    f32 = mybir.dt.float32
    f16 = mybir.dt.float16
    f8 = mybir.dt.float8e3
    nc = bacc.Bacc("TRN2", target_bir_lowering=False, debug=False,
                   num_devices=N_CORES)
    # main x, partition-major: [p, pass, k, t, c]; feature f = k*128 + p,
    # batch b = pass*4096 + t*512 + c.  Per (pass,k) DMA: 4 KB/partition runs.
    xk = nc.dram_tensor("xk", [KT, NP, NK, TPP, NB], f8,
                        kind="ExternalInput").ap()
    # remainder features 768..784: [p, batch]
    xrem = nc.dram_tensor("xrem", [DR, B_SH], f8, kind="ExternalInput").ap()
    # weights pre-arranged host-side: wk[p, k, m] = W_eff[k*128+p, m] / 2
    wk = nc.dram_tensor("wk", [KT, NK, H], f16, kind="ExternalInput").ap()
    wr = nc.dram_tensor("wr", [DR, H], f16, kind="ExternalInput").ap()
    w2t = nc.dram_tensor("w2t", [H, OUT], f16, kind="ExternalInput").ap()
    b1c = nc.dram_tensor("b1c", [H, 1], f32, kind="ExternalInput").ap()
    b2c = nc.dram_tensor("b2c", [OUT, 1], f32, kind="ExternalInput").ap()
    out = nc.dram_tensor("out", [OUT, B_SH], f32, kind="ExternalOutput").ap()

    with tile.TileContext(nc) as tc:
        with (
            tc.tile_pool(name="wpool", bufs=1) as wpool,
            tc.tile_pool(name="xpool", bufs=NP * NK) as xpool,
            tc.tile_pool(name="hpool", bufs=6) as hpool,
            tc.tile_pool(name="opool", bufs=NP) as opool,
            tc.tile_pool(name="ps", bufs=8, space="PSUM") as pspool,
        ):
            # Params on gpsimd's software DGE: it comes up before the HW
            # rings' cold start, so weights beat the first x block.
            wk_sb = wpool.tile([KT, NK, H], f16)
            nc.gpsimd.dma_start(wk_sb[:], wk[:])
            wr_sb = wpool.tile([DR, H], f16)
            nc.gpsimd.dma_start(wr_sb[:], wr[:])
            w2_sb = wpool.tile([H, OUT], f16)
            nc.gpsimd.dma_start(w2_sb[:], w2t[:])
            b1_sb = wpool.tile([H, 1], f32)
            nc.gpsimd.dma_start(b1_sb[:], b1c[:])
            b2_sb = wpool.tile([OUT, 1], f32)
            nc.gpsimd.dma_start(b2_sb[:], b2c[:])
            xr_sb = wpool.tile([DR, B_SH], f8)
            nc.gpsimd.dma_start(xr_sb[:], xrem[:])

            # x blocks alternate between the two HW DGE rings, emitted in
            # consumption order; all 12 resident (no buffer-reuse stalls).
            x_sb = []
            for pa in range(NP):
                for k in range(NK):
                    t_ = xpool.tile([KT, TPP * NB], f8)
                    eng = nc.sync if (pa * NK + k) % 2 == 0 else nc.scalar
                    eng.dma_start(
                        t_[:], xk[:, pa, k, :, :].rearrange("p t c -> p (t c)"))
                    x_sb.append(t_)

            # PE warm-up on a memset tile: no DMA dependency, so it runs from
            # queue boot (~6us) until the first x block lands (~10us),
            # holding the HAM at full clock for the real stream.
            warm_x = wpool.tile([KT, NB], f16)
            nc.vector.memset(warm_x[:], 0.0)
            warm_ps = pspool.tile([H, NB], f32, name="acc")
            for _ in range(NWARM):
                nc.tensor.matmul(warm_ps[:], lhsT=warm_x[:, 0:H],
                                 rhs=warm_x[:], start=True, stop=True)

            def epilogue(t, ps1, o_sb):
                # h = relu(ps + b1), alternating ACT/DVE; fp16 out
                h_sb = hpool.tile([H, NB], f16)
                if t % 2 == 0:
                    nc.scalar.activation(
                        h_sb[:], ps1[:],
                        mybir.ActivationFunctionType.Relu, bias=b1_sb[:])
                else:
                    nc.vector.tensor_scalar(
                        h_sb[:], ps1[:], b1_sb[:], 0.0,
                        mybir.AluOpType.add, mybir.AluOpType.max)
                # out.T[10, 512] = w2 @ h.T into the just-freed bank
                ps2 = pspool.tile([OUT, NB], f32, name="acc")
                nc.tensor.matmul(ps2[:], lhsT=w2_sb[:], rhs=h_sb[:],
                                 start=True, stop=True)
                # +b2 on the opposite engine into the pass output strip
                if t % 2 == 0:
                    nc.vector.tensor_scalar_add(
                        o_sb[:, t * NB:(t + 1) * NB], ps2[:], b2_sb[:])
                else:
                    nc.scalar.activation(
                        o_sb[:, t * NB:(t + 1) * NB], ps2[:],
                        mybir.ActivationFunctionType.Identity, bias=b2_sb[:])

            for pa in range(NP):
                ps_t = [pspool.tile([H, NB], f32, name="acc")
                        for t in range(TPP)]
                # k-outer: one stationary weight feeds 8 consecutive matmuls
                for k in range(NK):
                    xs = x_sb[pa * NK + k]
                    for t in range(TPP):
                        nc.tensor.matmul(
                            ps_t[t][:],
                            lhsT=wk_sb[:, k, :],
                            rhs=xs[:, t * NB:(t + 1) * NB],
                            start=(k == 0),
                            stop=False,
                        )
                o_sb = opool.tile([OUT, TPP * NB], f32)
                # remainder matmuls finish tiles one at a time; epilogues lag
                # two tiles so the PE never waits on a fresh relu (relu ~700ns
                # vs 390ns/matmul issue spacing)
                for t in range(TPP):
                    b0 = (pa * TPP + t) * NB
                    nc.tensor.matmul(
                        ps_t[t][:], lhsT=wr_sb[:],
                        rhs=xr_sb[:, b0:b0 + NB],
                        start=False, stop=True,
                    )
                    if t >= 2:
                        epilogue(t - 2, ps_t[t - 2], o_sb)
                epilogue(TPP - 2, ps_t[TPP - 2], o_sb)
                epilogue(TPP - 1, ps_t[TPP - 1], o_sb)
                # one store per pass on the sync ring (x triggers long done)
                nc.sync.dma_start(
                    out[:, pa * TPP * NB:(pa + 1) * TPP * NB], o_sb[:])

    nc.compile()
    _strip_redundant_ldweights(nc)
    return nc


def _get_nc():
    if "nc" not in _CACHE:
        _CACHE["nc"] = _build_nc()
    return _CACHE["nc"]


def _fold_weights(conv_w: np.ndarray, w1: np.ndarray) -> np.ndarray:
    """W_eff[784, 128]: h_pre = x @ W_eff  ==  conv(x) @ w1.T  (float64 accum)."""
    w1k = w1.reshape(H, 26, 26).transpose(1, 2, 0).astype(np.float64)  # [i,j,k]
    cw = conv_w.astype(np.float64)
    W = np.zeros((28, 28, H), np.float64)
    for di in range(3):
        for dj in range(3):
            W[di:di + 26, dj:dj + 26, :] += cw[di, dj] * w1k
    return W.reshape(D, H).astype(np.float32)


def make_in_maps(x, conv_w, w1, b1, w2, b2):
    f8 = ml_dtypes.float8_e3m4
    x = np.asarray(x, np.float32)
    weff = _fold_weights(np.asarray(conv_w, np.float32),
                         np.asarray(w1, np.float32)) * 0.5  # absorb x*2
    # wk[p, k, m] = weff[k*128+p, m]
    wk = np.ascontiguousarray(
        weff[:DM].reshape(NK, KT, H).transpose(1, 0, 2)).astype(np.float16)
    wr = np.ascontiguousarray(weff[DM:]).astype(np.float16)
    w2t = np.ascontiguousarray(np.asarray(w2, np.float32).T).astype(np.float16)
    b1c = np.ascontiguousarray(np.asarray(b1, np.float32).reshape(H, 1))
    b2c = np.ascontiguousarray(np.asarray(b2, np.float32).reshape(OUT, 1))
    in_maps = []
    for i in range(N_CORES):
        xs = (x[i * B_SH:(i + 1) * B_SH] * 2.0).astype(f8)  # [8192, 784]
        # main: [pass*4096 + t*512 + c, k*128 + p] -> [p, pass, k, t, c]
        xkv = np.ascontiguousarray(
            xs[:, :DM].reshape(NP, TPP, NB, NK, KT).transpose(4, 0, 3, 1, 2))
        xremv = np.ascontiguousarray(xs[:, DM:].T)           # [16, 8192]
        in_maps.append({"xk": xkv, "xrem": xremv, "wk": wk, "wr": wr,
                        "w2t": w2t, "b1c": b1c, "b2c": b2c})
    return in_maps


def kernel(x, conv_w, w1, b1, w2, b2):
    nc = _get_nc()
    in_maps = make_in_maps(x, conv_w, w1, b1, w2, b2)
    res = run_bass_kernel_spmd(nc, in_maps, list(range(N_CORES)))
    out = np.concatenate([res.results[i]["out"] for i in range(N_CORES)], axis=1)
    return np.ascontiguousarray(out.T)  # [65536, 10] float32
